# revision 43
# baseline (speedup 1.0000x reference)
"""Trainium2 Bass kernel for nn_AttentionBlock (B=8, C=512, H=W=64).

Sharding: data-parallel over batch. One batch element per NeuronCore,
8 cores, identical SPMD program, per-core inputs differ only in `x`.

Per-core pipeline (all activations [channels, n] with n = H*W = 4096):
  1. GroupNorm(32 groups) fp32 stats via bn_stats/bn_aggr (DVE), group
     reduction + broadcast via tiny PE matmuls with group masks, affine
     apply (split across DVE and ACT) fused with the bf16 downcast.
  2. Q/K 1x1 convs -> [C, N] bf16;  V conv -> [N, C] bf16 (transposed
     layout so V tiles are the stationary matmul operand in PV).
  3. Attention over 8 query blocks of 512 (software-pipelined):
       QK phase:  S'[j,i] = sum_c K[c,j] Q[c,i] per key tile j (PSUM),
                  E = exp(S'/sqrt(C)) on ACT (no max subtraction --
                  scores are O(6)), denominator accumulated on DVE.
       PV phase:  out'[c,i] += V[j,c]^T E[j,i]  (PSUM accum over j)
       normalize: ones-matmul row sum -> reciprocal -> K=1 broadcast
                  matmul -> out_norm = out' * r  (overlapped with the
                  NEXT block's QK phase so the PE never waits)
  4. proj conv + bias + fp32 residual, interleaved into the next
     block's PV phase; streamed to DRAM.
"""

import sys

import numpy as np

if "/opt/trn_rl_repo" not in sys.path:
    sys.path.insert(0, "/opt/trn_rl_repo")

B, C, HH, WW = 8, 512, 64, 64
N = HH * WW          # 4096
P = 128              # partitions
NPT = C // P         # 4 channel partition-tiles
NT = N // P          # 32 key tiles
NCH = N // 512       # 8 n-chunks / query blocks
GPP = 8              # groups per channel partition-tile (128/16)
CPG = 16             # channels per group
EPS = 1e-5
SCALE = float(1.0 / np.sqrt(C))

_CACHE = {}


def _build_program():
    import concourse.bacc as bacc
    import concourse.bass as bass
    import concourse.mybir as mybir
    from concourse import tile

    f32 = mybir.dt.float32
    bf16 = mybir.dt.bfloat16
    f8 = mybir.dt.float8e4
    DR = mybir.MatmulPerfMode.DoubleRow
    AF = mybir.ActivationFunctionType
    OP = mybir.AluOpType
    PSUM = bass.MemorySpace.PSUM

    nc = bacc.Bacc("TRN2", target_bir_lowering=False, debug=False,
                   enable_asserts=False)

    x_d = nc.dram_tensor("x", [C, N], f32, kind="ExternalInput")
    w_d = {
        nm: nc.dram_tensor(nm, [C, C], bf16, kind="ExternalInput")
        for nm in ("wqT", "wkT", "wvT", "wpT")
    }
    b_d = {
        nm: nc.dram_tensor(nm, [P, NPT], f32, kind="ExternalInput")
        for nm in ("bq", "bk", "bp", "gamma", "beta")
    }
    bvrow_d = nc.dram_tensor("bvrow", [1, C], f32, kind="ExternalInput")
    gmask_d = nc.dram_tensor("gmask", [P, GPP], f32, kind="ExternalInput")
    gmaskT_d = nc.dram_tensor("gmaskT", [GPP, P], f32, kind="ExternalInput")
    out_d = nc.dram_tensor("out", [C, N], f32, kind="ExternalOutput")

    with tile.TileContext(nc) as tc:
        from contextlib import ExitStack

        with ExitStack() as root:
            consts = root.enter_context(tc.tile_pool(name="consts", bufs=1))

            wsb = {}
            for nm in ("wpT",):
                tiles = []
                for c in range(NPT):
                    t = consts.tile([P, C], bf16, tag=f"{nm}{c}",
                                    name=f"{nm}{c}")
                    nc.sync.dma_start(t[:], w_d[nm][c * P:(c + 1) * P, :])
                    tiles.append(t)
                wsb[nm] = tiles
            bsb = {}
            for nm in ("bq", "bk", "bp", "gamma", "beta"):
                t = consts.tile([P, NPT], f32, tag=nm, name=nm)
                nc.sync.dma_start(t[:], b_d[nm][:, :])
                bsb[nm] = t
            bvrow = consts.tile([1, C], f32, tag="bvrow", name="bvrow")
            nc.sync.dma_start(bvrow[:], bvrow_d[:, :])
            gmask = consts.tile([P, GPP], f32, tag="gmask", name="gmask")
            nc.sync.dma_start(gmask[:], gmask_d[:, :])
            gmaskT = consts.tile([GPP, P], f32, tag="gmaskT", name="gmaskT")
            nc.sync.dma_start(gmaskT[:], gmaskT_d[:, :])
            ones128 = consts.tile([P, 1], f32, tag="ones128", name="ones128")
            nc.vector.memset(ones128[:], 1.0)
            ones1 = consts.tile([1, P], f32, tag="ones1", name="ones1")
            nc.vector.memset(ones1[:], 1.0)
            ones128b = consts.tile([P, 1], bf16, tag="ones128b",
                                   name="ones128b")
            nc.vector.memset(ones128b[:], 1.0)
            ones1b = consts.tile([1, P], bf16, tag="ones1b", name="ones1b")
            nc.vector.memset(ones1b[:], 1.0)
            eps_t = consts.tile([P, 1], f32, tag="eps", name="eps")
            nc.vector.memset(eps_t[:], EPS)
            # constant shift inside exp keeps E within fp8e4 range; it
            # cancels exactly in the softmax normalization
            shift_t = consts.tile([P, 1], f32, tag="shift", name="shift")
            nc.vector.memset(shift_t[:], -3.0)
            bvb = consts.tile([P, C], f32, tag="bvb", name="bvb")

            # h tiles (GroupNorm output, bf16); slots reused by out_sb
            # (same tag) once the QKV convs are done with h.
            hpool = root.enter_context(tc.tile_pool(name="hpool", bufs=NPT))

            # persistent activation pools must sit below wqkv on the
            # stack allocator so wqkv's release frees space for the
            # attention-phase pools
            qkpool = root.enter_context(tc.tile_pool(name="qkpool",
                                                     bufs=2))
            vpool = root.enter_context(tc.tile_pool(name="vpool",
                                                    bufs=NT // 2))
            # fp8 DoubleRow slabs: dim 1 is the 2-way contraction
            # interleave. q/k pair channels (c, c+128) within each half
            # of the channel range; v pairs adjacent key tiles.
            q8 = [qkpool.tile([P, 2, N], f8, tag="q8", name=f"q8_{i}")
                  for i in range(2)]
            k8 = [qkpool.tile([P, 2, N], f8, tag="k8", name=f"k8_{i}")
                  for i in range(2)]
            v8 = [vpool.tile([P, 2, C], f8, tag="v8", name=f"v8_{i}")
                  for i in range(NT // 2)]

            # q/k/v conv weights live only until the convs are done; their
            # SBUF space is then reclaimed for the attention-phase pools.
            qkv_scope = ExitStack()
            wqkv = qkv_scope.enter_context(
                tc.tile_pool(name="wqkv", bufs=1))
            for nm in ("wqT", "wkT", "wvT"):
                tiles = []
                for c in range(NPT):
                    t = wqkv.tile([P, C], bf16, tag=f"{nm}{c}",
                                  name=f"{nm}{c}")
                    nc.sync.dma_start(t[:], w_d[nm][c * P:(c + 1) * P, :])
                    tiles.append(t)
                wsb[nm] = tiles

            # GroupNorm is folded into the convs: with h = x*a + b
            # (a, b per channel), q = Wq h + bq = (Wq diag(a)) x + (Wq b
            # + bq). So the convs read a plain bf16 cast of x, the conv
            # weights get scaled by a in place, and the bias vectors are
            # computed with tiny matmuls. This takes the whole affine
            # apply pass off the startup critical path.
            hs = []      # bf16 casts of x
            a_ps = []    # per-channel scale
            b_bfs = []   # per-channel shift (bf16, contraction operand)
            with tc.tile_pool(name="psA", bufs=2, space=PSUM) as psA, \
                 tc.tile_pool(name="xpool", bufs=2) as xpool, \
                 tc.tile_pool(name="smalls", bufs=8) as smalls:
                # keep the PE busy (and its HAM clock-gate warm) during
                # the GroupNorm stats latency with throwaway matmuls --
                # nothing reads `warm`, the PE is otherwise idle here.
                # Emitted in chunks between ptile iterations so each
                # ptile's tiny group-stats matmuls aren't queued behind
                # the whole warm stream on the in-order PE.
                warm = psA.tile([P, 512], f32, tag="warm", bufs=1,
                                name="warm")

                def emit_warm(n):
                    for _ in range(n):
                        nc.tensor.matmul(warm[:], wsb["wpT"][0][:, 0:P],
                                         wsb["wpT"][0][:, 0:512])

                emit_warm(30)
                for p in range(NPT):
                    x_p = xpool.tile([P, N], f32, tag="x", name=f"xg{p}")
                    engs = [nc.sync, nc.gpsimd, nc.scalar]
                    half = N // 2
                    for hh in range(2):
                        engs[(2 * p + hh) % 3].dma_start(
                            x_p[:, hh * half:(hh + 1) * half],
                            x_d[p * P:(p + 1) * P, hh * half:(hh + 1) * half])
                    h_p = hpool.tile([P, N], bf16, tag="hb", name=f"h{p}")
                    # stats split across engines: ACT squares+sums half B
                    # (h_p as scratch), then casts half B with the plain
                    # sum as accum; DVE does bn_stats on half A + cast A
                    st4 = smalls.tile([P, 2], f32, tag="st4", name=f"st4{p}")
                    nc.scalar.activation(h_p[:, half:N], x_p[:, half:N],
                                         AF.Square,
                                         accum_out=st4[:, 1:2])
                    nc.scalar.activation(h_p[:, half:N], x_p[:, half:N],
                                         AF.Copy, accum_out=st4[:, 0:1])
                    nc.vector.tensor_copy(h_p[:, 0:half], x_p[:, 0:half])
                    bns = smalls.tile([P, 4 * 6], f32, tag="bns",
                                      name=f"bns{p}")
                    for s in range(4):
                        nc.vector.bn_stats(bns[:, s * 6:(s + 1) * 6],
                                           x_p[:, s * 512:(s + 1) * 512])
                    cst = smalls.tile([P, 2], f32, tag="cst", name=f"cst{p}")
                    nc.vector.bn_aggr(cst[:], bns[:])
                    stats = smalls.tile([P, 2], f32, tag="stats",
                                        name=f"stats{p}")
                    # s1 = mean_a*half + s1_b
                    nc.vector.scalar_tensor_tensor(
                        stats[:, 0:1], cst[:, 0:1], float(half),
                        st4[:, 0:1], OP.mult, OP.add)
                    # m2_a = mean_a^2 + var_a ; s2 = m2_a*half + s2_b
                    m2a = smalls.tile([P, 1], f32, tag="m2a", name=f"m2a{p}")
                    nc.vector.scalar_tensor_tensor(
                        m2a[:], cst[:, 0:1], cst[:, 0:1], cst[:, 1:2],
                        OP.mult, OP.add)
                    nc.vector.scalar_tensor_tensor(
                        stats[:, 1:2], m2a[:], float(half), st4[:, 1:2],
                        OP.mult, OP.add)
                    gst = psA.tile([GPP, 2], f32, tag="ps", name=f"gst{p}")
                    nc.tensor.matmul(gst[:], gmask[:], stats[:])
                    mu = smalls.tile([GPP, 2], f32, tag="mu", name=f"mu{p}")
                    nc.vector.tensor_scalar_mul(mu[:], gst[:],
                                                1.0 / (CPG * N))
                    musq = smalls.tile([GPP, 1], f32, tag="musq",
                                       name=f"musq{p}")
                    nc.vector.tensor_tensor(musq[:], mu[:, 0:1], mu[:, 0:1],
                                            OP.mult)
                    var = smalls.tile([GPP, 1], f32, tag="var",
                                      name=f"var{p}")
                    nc.vector.tensor_tensor(var[:], mu[:, 1:2], musq[:],
                                            OP.subtract)
                    sq = smalls.tile([GPP, 1], f32, tag="sq", name=f"sq{p}")
                    nc.scalar.activation(sq[:], var[:], AF.Sqrt,
                                         bias=eps_t[:GPP, 0:1])
                    rsqmu = smalls.tile([GPP, 2], f32, tag="rsqmu",
                                        name=f"rsqmu{p}")
                    nc.vector.reciprocal(rsqmu[:, 0:1], sq[:])
                    nc.vector.tensor_copy(rsqmu[:, 1:2], mu[:, 0:1])
                    bc = psA.tile([P, 2], f32, tag="ps", name=f"bc{p}")
                    nc.tensor.matmul(bc[:], gmaskT[:], rsqmu[:])
                    emit_warm(20)
                    a_p = smalls.tile([P, 1], f32, tag="a", name=f"a{p}")
                    t_p = smalls.tile([P, 1], f32, tag="t", name=f"t{p}")
                    b_p = smalls.tile([P, 1], f32, tag="b", name=f"b{p}")
                    nc.vector.tensor_tensor(a_p[:], bsb["gamma"][:, p:p + 1],
                                            bc[:, 0:1], OP.mult)
                    nc.vector.tensor_tensor(t_p[:], bc[:, 1:2], a_p[:],
                                            OP.mult)
                    nc.vector.tensor_tensor(b_p[:], bsb["beta"][:, p:p + 1],
                                            t_p[:], OP.subtract)
                    b_bf = smalls.tile([P, 1], bf16, tag="bbf",
                                       name=f"bbf{p}")
                    nc.vector.tensor_copy(b_bf[:], b_p[:])
                    hs.append(h_p)
                    a_ps.append(a_p)
                    b_bfs.append(b_bf)

                # bias vectors: bias_q = Wq b + bq (column layout), and
                # the v bias as a partition-broadcast row
                biasqk = {}
                for nm, bias in (("wqT", "bq"), ("wkT", "bk")):
                    bt = consts.tile([P, NPT], f32, tag=f"bias{nm}",
                                     name=f"bias{nm}")
                    for o in range(NPT):
                        bps = psA.tile([P, 1], f32, tag="ps",
                                       name=f"bps{nm}{o}")
                        for c in range(NPT):
                            nc.tensor.matmul(
                                bps[:], wsb[nm][c][:, o * P:(o + 1) * P],
                                b_bfs[c][:], start=(c == 0),
                                stop=(c == NPT - 1))
                        nc.vector.tensor_scalar(bt[:, o:o + 1], bps[:],
                                                bsb[bias][:, o:o + 1], None,
                                                OP.add)
                    biasqk[nm] = bt
                brow_ps = psA.tile([1, C], f32, tag="ps", name="brow_ps")
                for c in range(NPT):
                    nc.tensor.matmul(brow_ps[:], b_bfs[c][:],
                                     wsb["wvT"][c][:, :], start=(c == 0),
                                     stop=(c == NPT - 1))
                brow_sb = smalls.tile([1, C], f32, tag="brow", name="brow")
                nc.vector.tensor_tensor(brow_sb[:], brow_ps[:], bvrow[:],
                                        OP.add)
                bvb_ps = psA.tile([P, C], f32, tag="ps", name="bvb_ps")
                nc.tensor.matmul(bvb_ps[:], ones1[:], brow_sb[:])
                nc.vector.tensor_copy(bvb[:], bvb_ps[:])
                # scale conv weights by a in place (contraction-side)
                for nm in ("wqT", "wkT", "wvT"):
                    for c in range(NPT):
                        nc.vector.tensor_scalar_mul(wsb[nm][c][:],
                                                    wsb[nm][c][:],
                                                    a_ps[c][:])

            # ---------------- QKV convs ----------------
            with tc.tile_pool(name="psC", bufs=8, space=PSUM) as psC:
                for nch in range(NCH):
                    sl = slice(nch * 512, (nch + 1) * 512)
                    for o in range(NPT):
                        kps = psC.tile([P, 512], f32, tag="c",
                                       name=f"kps{nch}_{o}")
                        for c in range(NPT):
                            nc.tensor.matmul(
                                kps[:], wsb["wkT"][c][:, o * P:(o + 1) * P],
                                hs[c][:, sl], start=(c == 0),
                                stop=(c == NPT - 1))
                        nc.vector.tensor_scalar(k8[o // 2][:, o % 2, sl],
                                                kps[:],
                                                biasqk["wkT"][:, o:o + 1],
                                                None, OP.add)
                    for o in range(NPT):
                        qps = psC.tile([P, 512], f32, tag="c",
                                       name=f"qps{nch}_{o}")
                        for c in range(NPT):
                            nc.tensor.matmul(
                                qps[:], wsb["wqT"][c][:, o * P:(o + 1) * P],
                                hs[c][:, sl], start=(c == 0),
                                stop=(c == NPT - 1))
                        nc.scalar.activation(q8[o // 2][:, o % 2, sl],
                                             qps[:], AF.Identity,
                                             bias=biasqk["wqT"][:, o:o + 1])
                    for t in range(4):
                        nt = nch * 4 + t
                        vps = psC.tile([P, 512], f32, tag="c",
                                       name=f"vps{nt}")
                        for c in range(NPT):
                            nc.tensor.matmul(
                                vps[:], hs[c][:, nt * P:(nt + 1) * P],
                                wsb["wvT"][c][:, :], start=(c == 0),
                                stop=(c == NPT - 1))
                        nc.vector.scalar_tensor_tensor(
                            v8[nt // 2][:, nt % 2, :], vps[:], 1.0, bvb[:],
                            OP.mult, OP.add)

            # ------------- attention + interleaved proj -------------
            qkv_scope.close()  # release the q/k/v weight SBUF space
            out_sb = [hpool.tile([P, N], bf16, tag="hb", name=f"osb{i}")
                      for i in range(NPT)]
            with tc.tile_pool(name="psS", bufs=2, space=PSUM) as psS, \
                 tc.tile_pool(name="psO", bufs=NPT, space=PSUM) as psO, \
                 tc.tile_pool(name="att", bufs=4) as att, \
                 tc.tile_pool(name="epool", bufs=NT // 2 + 4) as epool, \
                 tc.tile_pool(name="fin", bufs=4) as fin:
                psM = psS  # ds/rb/proj psum tiles share the psS slots

                state = {}

                def emit_qk(ib, mid_cb=None):
                    isl = slice(ib * 512, (ib + 1) * 512)
                    # prefetch the residual x tiles this block's proj needs
                    xts = []
                    for o in range(NPT):
                        x_t = fin.tile([P, 512], f32, tag="xr",
                                       name=f"xr{ib}_{o}")
                        nc.gpsimd.dma_start(x_t[:],
                                            x_d[o * P:(o + 1) * P, isl])
                        xts.append(x_t)
                    state[ib] = {"xts": xts}
                    dacc = att.tile([P, 512], f32, tag="dacc", bufs=2,
                                    name=f"dacc{ib}")
                    es = []
                    for t in range(NT // 2):
                        if t == 7 and mid_cb is not None:
                            mid_cb()
                        # one 2-bank PSUM tile holds the scores of two
                        # key tiles; one ACTIVATE exps both (the +352cyc
                        # ACT instruction overhead amortizes over 1024)
                        sps = psS.tile([P, 2, 512], f32, tag="s",
                                       name=f"s{ib}_{t}")
                        for r in range(2):
                            j = 2 * t + r
                            for h in range(2):
                                nc.tensor.matmul(
                                    sps[:, r, :],
                                    k8[h][:, :, j * P:(j + 1) * P],
                                    q8[h][:, :, isl], start=(h == 0),
                                    stop=(h == 1), perf_mode=DR)
                        e_t = epool.tile([P, 2, 512], f8, tag="e",
                                         name=f"e{ib}_{t}")
                        nc.scalar.activation(e_t[:], sps[:], AF.Exp,
                                             scale=SCALE,
                                             bias=shift_t[:, 0:1])
                        es.append(e_t)
                        if t == 0:
                            nc.vector.tensor_copy(dacc[:], e_t[:, 0, :])
                        else:
                            nc.vector.tensor_tensor(dacc[:], dacc[:],
                                                    e_t[:, 0, :], OP.add)
                        nc.vector.tensor_tensor(dacc[:], dacc[:],
                                                e_t[:, 1, :], OP.add)
                    state[ib]["es"] = es
                    state[ib]["dacc"] = dacc

                def emit_dsum(ib):
                    # row sums -> reciprocal (runs on DVE during the next
                    # block's QK phase)
                    dacc_bf = att.tile([P, 512], bf16, tag="daccb",
                                       bufs=2, name=f"daccb{ib}")
                    nc.vector.tensor_copy(dacc_bf[:], state[ib]["dacc"])
                    dsum = psM.tile([1, 512], f32, tag="s", name=f"ds{ib}")
                    nc.tensor.matmul(dsum[:], ones128b[:], dacc_bf[:])
                    r_sb = att.tile([1, 512], bf16, tag="r", bufs=2,
                                    name=f"r{ib}")
                    with nc.allow_low_precision(
                            reason="softmax denom reciprocal in bf16"):
                        nc.vector.reciprocal(r_sb[:], dsum[:])
                    state[ib]["r"] = r_sb

                def emit_norm(ib):
                    # broadcast 1/d across partitions and normalize out'
                    isl = slice(ib * 512, (ib + 1) * 512)
                    rb_ps = psM.tile([P, 512], f32, tag="s", name=f"rb{ib}")
                    nc.tensor.matmul(rb_ps[:], ones1b[:], state[ib]["r"])
                    rb_sb = att.tile([P, 512], f32, tag="rb", bufs=2,
                                     name=f"rbs{ib}")
                    nc.vector.tensor_copy(rb_sb[:], rb_ps[:])
                    for c in range(NPT):
                        nc.vector.scalar_tensor_tensor(
                            out_sb[c][:, isl], state[ib]["ops"][c], 1.0,
                            rb_sb[:], OP.mult, OP.mult)

                def emit_proj_group(ib, o):
                    isl = slice(ib * 512, (ib + 1) * 512)
                    pps = psM.tile([P, 512], f32, tag="s",
                                   name=f"pps{ib}_{o}")
                    for c in range(NPT):
                        nc.tensor.matmul(
                            pps[:], wsb["wpT"][c][:, o * P:(o + 1) * P],
                            out_sb[c][:, isl], start=(c == 0),
                            stop=(c == NPT - 1))
                    res = fin.tile([P, 512], f32, tag="res",
                                   name=f"res{ib}_{o}")
                    nc.vector.scalar_tensor_tensor(
                        res[:], pps[:], bsb["bp"][:, o:o + 1],
                        state[ib]["xts"][o], OP.add, OP.add)
                    nc.sync.dma_start(out_d[o * P:(o + 1) * P, isl], res[:])

                def emit_pv(ib, prev):
                    ops = [psO.tile([P, 512], f32, tag="o",
                                    name=f"op{ib}_{c}") for c in range(NPT)]
                    state[ib]["ops"] = ops
                    for t in range(NT // 2):
                        # the row-sum matmul goes here, two pair-groups
                        # into PV, so the PE never waits on the dacc
                        # chain (exp tail + adds + bf16 cast, ~2.5us)
                        if t == 2:
                            emit_dsum(ib)
                        # spread the previous block's proj through PV
                        if prev is not None and t % 4 == 2:
                            emit_proj_group(prev, t // 4)
                        e_t = state[ib]["es"][t]
                        for c in range(NPT):
                            nc.tensor.matmul(
                                ops[c][:], v8[t][:, :, c * P:(c + 1) * P],
                                e_t[:], start=(t == 0),
                                stop=(t == NT // 2 - 1), perf_mode=DR)

                for ib in range(NCH):
                    if ib > 0:
                        emit_qk(ib, mid_cb=(lambda p=ib - 1: emit_norm(p)))
                    else:
                        emit_qk(ib)
                    emit_pv(ib, ib - 1 if ib > 0 else None)
                last = NCH - 1
                emit_norm(last)
                for o in range(NPT):
                    emit_proj_group(last, o)

    nc.compile()
    return nc


def _get_program():
    if "nc" not in _CACHE:
        _CACHE["nc"] = _build_program()
    return _CACHE["nc"]


def _make_in_maps(inputs):
    import ml_dtypes

    bf = ml_dtypes.bfloat16
    f32 = np.float32

    def wT(w):
        return np.ascontiguousarray(np.asarray(w, dtype=f32).T).astype(bf)

    def colmaj(v):
        # [512] -> [128, 4] with out[p, t] = v[t*128 + p]
        return np.ascontiguousarray(
            np.asarray(v, dtype=f32).reshape(NPT, P).T)

    gm = np.zeros((P, GPP), f32)
    gm[np.arange(P), np.arange(P) // CPG] = 1.0
    common = {
        "wqT": wT(inputs["wq"]),
        "wkT": wT(inputs["wk"]),
        "wvT": wT(inputs["wv"]),
        "wpT": wT(inputs["wp"]),
        "bq": colmaj(inputs["bq"]),
        "bk": colmaj(inputs["bk"]),
        "bp": colmaj(inputs["bp"]),
        "gamma": colmaj(inputs["gn_gamma"]),
        "beta": colmaj(inputs["gn_beta"]),
        "bvrow": np.asarray(inputs["bv"], dtype=f32).reshape(1, C),
        "gmask": gm,
        "gmaskT": np.ascontiguousarray(gm.T),
    }
    x = np.asarray(inputs["x"], dtype=f32).reshape(B, C, N)
    return [dict(common, x=np.ascontiguousarray(x[i])) for i in range(B)]


def run(inputs, trace=False):
    """Returns (output [B, C, H, W] fp32, BassKernelResults)."""
    from concourse import bass_utils

    nc = _get_program()
    in_maps = _make_in_maps(inputs)
    res = bass_utils.run_bass_kernel_spmd(nc, in_maps,
                                          core_ids=list(range(B)),
                                          trace=trace)
    out = np.stack([res.results[i]["out"] for i in range(B)], axis=0)
    return out.reshape(B, C, HH, WW).astype(np.float32), res


def kernel(**inputs):
    out, _ = run(inputs, trace=False)
    return out
